# revision 11
# baseline (speedup 1.0000x reference)
"""Trainium2 Bass kernel for nn_Conv1d_NN (kNN + strided conv).

Math (per batch b):
    dist[t,s]  = ||x[:,t]||^2 + ||x[:,s]||^2 - 2 x[:,t].x[:,s]
    idx[t,:]   = top-8 smallest dist (self first), sorted ascending
    out[o,t]   = sum_{j,c} w[o,c,j] * x[c, idx[t,j]] + b[o]

Device strategy (data-parallel, 2 batches per core on 8 cores):
  - score[t,s] = 2 dot - ||x_s||^2 (row-constant shift of -dist preserves
    per-row ranking), computed at ~fp32 precision with PAIRED bf16
    matmuls: x = xh + xl (two bf16 planes, host-split), then
        MM1: [xh;xl]^T [2xh;2xl]  (xh.2xh + xl.2xl)
        MM2: [xh;xl]^T [2xl;2xh]  (xh.2xl + xl.2xh)
        MM3: ones3^T  [-nhi;-nmid;-nlo]  (norm as 3 bf16 rows)
    accumulated in one fp32 PSUM bank per 512-chunk. bf16 streams at
    1 cycle/row vs fp32's 4, so this is ~2x cheaper than one fp32 MM
    and ~fp32-accurate (dropped residual ~2^-18, ~10 wrong neighbor
    indices out of 262144 -> rel err ~8e-3, tolerance 2e-2).
  - DVE max/max_index -> top-8 values + column indices per token
    (exact fp32 compare; token tiles are CONTIGUOUS 128-token slices).
  - y[t,(j,o)] = sum_c x[c,t] w[o,c,j] via the same [xh;xl] lhsT against
    a [128,512] bf16 weight block (w rows duplicated in both planes).
    Bias is added on the host during the gather.
  - Outputs: y (all taps, all tokens) and the top-8 index table.

The final rank-indexed 8-way gather+sum runs on the host: this
container's runtime has no working data-dependent DMA (HIPI gpsimd
ucode excluded, DynamicAP indirect DMA generates broken descriptors),
so the O(T*K*C) permutation+sum is applied to the device-computed
y/idx tensors host-side. All matmul FLOPs (distance matrix + conv) and
the top-k run on device.
"""

import sys
import numpy as np

if "/opt/trn_rl_repo" not in sys.path:
    sys.path.insert(0, "/opt/trn_rl_repo")

B, C, T, K, OUT_C = 16, 64, 2048, 8, 64
NCORES = 8
BPC = B // NCORES  # batches per core
RT = T // 128      # 16 row tiles of 128 tokens
NF = T // 512      # 4 column chunks of 512

_CACHE = {}


def build_nc():
    import concourse.bacc as bacc
    import concourse.tile as tile
    import concourse.mybir as mybir

    dt = mybir.dt
    f32 = dt.float32
    bf16 = dt.bfloat16

    nc = bacc.Bacc(
        "TRN2", target_bir_lowering=False, debug=False, num_devices=NCORES
    )
    xp_d = nc.dram_tensor("xpair", [BPC, 128, T], bf16, kind="ExternalInput").ap()
    ra_d = nc.dram_tensor("ra", [BPC, 128, T], bf16, kind="ExternalInput").ap()
    rb_d = nc.dram_tensor("rb", [BPC, 128, T], bf16, kind="ExternalInput").ap()
    nm_d = nc.dram_tensor("nrmf", [BPC, 1, T], f32, kind="ExternalInput").ap()
    nb_d = nc.dram_tensor("nrmb", [BPC, 3, T], bf16, kind="ExternalInput").ap()
    wall_d = nc.dram_tensor("wall", [128, K * OUT_C], bf16, kind="ExternalInput").ap()
    y_d = nc.dram_tensor("yout", [BPC, T, K * OUT_C], f32, kind="ExternalOutput").ap()
    gi_d = nc.dram_tensor("gidx", [BPC, T, K], dt.uint16, kind="ExternalOutput").ap()

    with tile.TileContext(nc) as tc:
        with (
            tc.tile_pool(name="const", bufs=1) as constp,
            tc.tile_pool(name="xio", bufs=2) as xio,
            tc.tile_pool(name="scoresp", bufs=3) as scp,
            tc.tile_pool(name="small", bufs=3) as smp,
            tc.tile_pool(name="yio", bufs=3) as yp,
            tc.tile_pool(name="pd", bufs=6, space="PSUM") as pdp,
            tc.tile_pool(name="py", bufs=2, space="PSUM") as pyp,
        ):
            wall_sb = constp.tile([128, K * OUT_C], bf16)
            nc.sync.dma_start(wall_sb[:], wall_d[:])
            ones3 = constp.tile([3, 128], bf16)
            nc.gpsimd.memset(ones3[:], 1.0)

            H = T // 2
            xps, ras, rbs, nms, nmbs = [], [], [], [], []
            for b in range(BPC):
                # split the big inputs into halves so the first tile's
                # matmuls (chunks 2,3 = second half) start after ~1/2 the
                # input DMA; prefetch BOTH batches before any compute.
                xpB = xio.tile([128, H], bf16, tag="xpB", name=f"xpB{b}")
                raB = xio.tile([128, H], bf16, tag="raB", name=f"raB{b}")
                xpA = xio.tile([128, H], bf16, tag="xpA", name=f"xpA{b}")
                raA = xio.tile([128, H], bf16, tag="raA", name=f"raA{b}")
                rbA = xio.tile([128, H], bf16, tag="rbA", name=f"rbA{b}")
                rbB = xio.tile([128, H], bf16, tag="rbB", name=f"rbB{b}")
                nm = xio.tile([3, T], bf16, tag="nm", name=f"nm{b}")
                nr = xio.tile([1, T], f32, tag="nr", name=f"nr{b}")
                nc.sync.dma_start(xpA[:], xp_d[b, :, 0:H])
                nc.sync.dma_start(xpB[:], xp_d[b, :, H:T])
                nc.sync.dma_start(rbA[:], rb_d[b, :, 0:H])
                nc.sync.dma_start(raB[:], ra_d[b, :, H:T])
                nc.sync.dma_start(raA[:], ra_d[b, :, 0:H])
                nc.sync.dma_start(rbB[:], rb_d[b, :, H:T])
                nc.sync.dma_start(nm[:], nb_d[b])
                nc.sync.dma_start(nr[:], nm_d[b])
                nmb = xio.tile([128, H], f32, tag="nmb", name=f"nmb{b}")
                nc.gpsimd.partition_broadcast(nmb[:], nr[:, H:])
                xps.append((xpA, xpB))
                ras.append((raA, raB))
                rbs.append((rbA, rbB))
                nms.append(nm)
                nmbs.append(nmb)

            for b in range(BPC):
                (xpA, xpB), (raA, raB), (rbA, rbB) = xps[b], ras[b], rbs[b]
                nm, nmb = nms[b], nmbs[b]
                for rt in range(RT):
                    ts = slice((rt * 128) % H, (rt * 128) % H + 128)
                    lhs = (xpA if rt < RT // 2 else xpB)[:, ts]
                    lhs2 = (rbA if rt < RT // 2 else rbB)[:, ts]
                    scores = scp.tile([128, T], f32, tag="scores", name=f"sc{b}_{rt}")
                    # chunks 2,3 first (no PE norm matmul; gpsimd adds -norm)
                    for nf in (2, 3, 0, 1):
                        cs = slice(nf * 512, (nf + 1) * 512)
                        hs = slice((nf * 512) % H, (nf * 512) % H + 512)
                        rav = (raA if nf < 2 else raB)[:, hs]
                        xpv = (xpA if nf < 2 else xpB)[:, hs]
                        pd = pdp.tile([128, 512], f32, tag="pd", name=f"pd{b}_{rt}_{nf}")
                        if nf >= 2:
                            nc.tensor.matmul(pd[:], lhs, rav, start=True, stop=False)
                            nc.tensor.matmul(pd[:], lhs2, xpv, start=False, stop=True)
                        else:
                            nc.tensor.matmul(pd[:], lhs, rav, start=True, stop=False)
                            nc.tensor.matmul(pd[:], lhs2, xpv, start=False, stop=False)
                            nc.tensor.matmul(
                                pd[:], ones3[:], nm[:, cs], start=False, stop=True
                            )
                        nc.scalar.copy(scores[:, cs], pd[:])
                        if nf == 3:
                            nc.gpsimd.tensor_add(
                                scores[:, H:], scores[:, H:], nmb[:]
                            )

                    vals = smp.tile([128, 8], f32, tag="vals", name=f"v{b}_{rt}")
                    nc.vector.max(vals[:], scores[:])
                    gall = smp.tile([128, 8], dt.uint16, tag="gall", name=f"g{b}_{rt}")
                    nc.vector.max_index(gall[:], vals[:], scores[:])
                    nc.sync.dma_start(gi_d[b, rt * 128 : (rt + 1) * 128, :], gall[:])

                    py = pyp.tile([128, 512], f32, tag="py", name=f"py{b}_{rt}")
                    nc.tensor.matmul(py[:], lhs, wall_sb[:])
                    ysb = yp.tile([128, 512], f32, tag="ysb", name=f"y{b}_{rt}")
                    nc.scalar.copy(ysb[:], py[:])
                    nc.sync.dma_start(y_d[b, rt * 128 : (rt + 1) * 128, :], ysb[:])

    nc.compile()
    return nc


def _get_nc():
    if "nc" not in _CACHE:
        _CACHE["nc"] = build_nc()
    return _CACHE["nc"]


def host_inputs(x, w, b):
    """Per-core input maps from full inputs."""
    import ml_dtypes

    bf = ml_dtypes.bfloat16
    x = np.asarray(x, dtype=np.float32)
    w = np.asarray(w, dtype=np.float32)
    b = np.asarray(b, dtype=np.float32)

    xh = x.astype(bf).astype(np.float32)
    xl = (x - xh).astype(bf).astype(np.float32)
    norm = (x.astype(np.float64) ** 2).sum(axis=1).astype(np.float32)  # [B, T]
    nhi = norm.astype(bf).astype(np.float32)
    nmid = (norm - nhi).astype(bf).astype(np.float32)
    nlo = (norm - nhi - nmid).astype(bf).astype(np.float32)

    xpair = np.concatenate([xh, xl], axis=1).astype(bf)           # [B, 128, T]
    ra = np.concatenate([2 * xh, 2 * xl], axis=1).astype(bf)      # [B, 128, T]
    rb = np.concatenate([2 * xl, 2 * xh], axis=1).astype(bf)      # [B, 128, T]
    nrmf = (-norm)[:, None, :].astype(np.float32)                 # [B, 1, T]
    nrmb = np.stack([-nhi, -nmid, -nlo], axis=1).astype(bf)       # [B, 3, T]

    wr = w.transpose(1, 2, 0).reshape(C, K * OUT_C)               # [c, j*64+o]
    wall = np.concatenate([wr, wr], axis=0).astype(bf)            # [128, 512]

    return [
        {
            "xpair": np.ascontiguousarray(xpair[i * BPC : (i + 1) * BPC]),
            "ra": np.ascontiguousarray(ra[i * BPC : (i + 1) * BPC]),
            "rb": np.ascontiguousarray(rb[i * BPC : (i + 1) * BPC]),
            "nrmf": np.ascontiguousarray(nrmf[i * BPC : (i + 1) * BPC]),
            "nrmb": np.ascontiguousarray(nrmb[i * BPC : (i + 1) * BPC]),
            "wall": wall,
        }
        for i in range(NCORES)
    ]


def kernel(x, w, b):
    from concourse.bass_utils import run_bass_kernel_spmd

    nc = _get_nc()
    in_maps = host_inputs(x, w, b)
    res = run_bass_kernel_spmd(nc, in_maps, list(range(NCORES)))

    b32 = np.asarray(b, dtype=np.float32)
    out = np.empty((B, OUT_C, T), np.float32)
    jj = np.arange(K, dtype=np.int64)[None, :]
    for i in range(NCORES):
        yv = res.results[i]["yout"]    # [BPC, T, K*OUT_C]
        gi = res.results[i]["gidx"]    # [BPC, T, K] u16
        for bb in range(BPC):
            idx = gi[bb].astype(np.int64)                 # [T, K]
            yr = yv[bb].reshape(T, K, OUT_C)              # [s, j, o]
            gathered = yr[idx, jj, :]                     # [T, K, OUT_C]
            out[i * BPC + bb] = gathered.sum(1).T + b32[:, None]
    return out.astype(np.float32)


# revision 13
# speedup vs baseline: 1.0308x; 1.0308x over previous
"""Trainium2 Bass kernel for nn_Conv1d_NN (kNN + strided conv).

Math (per batch b):
    dist[t,s]  = ||x[:,t]||^2 + ||x[:,s]||^2 - 2 x[:,t].x[:,s]
    idx[t,:]   = top-8 smallest dist (self first), sorted ascending
    out[o,t]   = sum_{j,c} w[o,c,j] * x[c, idx[t,j]] + b[o]

Device strategy (data-parallel, 2 batches per core on 8 cores):
  - score[t,s] = 2 dot - ||x_s||^2 (row-constant shift of -dist preserves
    per-row ranking), computed at ~fp32 precision with PAIRED bf16
    matmuls: x = xh + xl (two bf16 planes, host-split), then
        MM1: [xh;xl]^T [2xh;2xl]  (xh.2xh + xl.2xl)
        MM2: [xh;xl]^T [2xl;2xh]  (xh.2xl + xl.2xh)
        MM3: ones3^T  [-nhi;-nmid;-nlo]  (norm as 3 bf16 rows)
    accumulated in one fp32 PSUM bank per 512-chunk. bf16 streams at
    1 cycle/row vs fp32's 4, so this is ~2x cheaper than one fp32 MM
    and ~fp32-accurate (dropped residual ~2^-18, ~10 wrong neighbor
    indices out of 262144 -> rel err ~8e-3, tolerance 2e-2).
  - DVE max/max_index -> top-8 values + column indices per token
    (exact fp32 compare; token tiles are CONTIGUOUS 128-token slices).
  - y[t,(j,o)] = sum_c x[c,t] w[o,c,j] via the same [xh;xl] lhsT against
    a [128,512] bf16 weight block (w rows duplicated in both planes).
    Bias is added on the host during the gather.
  - Outputs: y (all taps, all tokens) and the top-8 index table.

The final rank-indexed 8-way gather+sum runs on the host: this
container's runtime has no working data-dependent DMA (HIPI gpsimd
ucode excluded, DynamicAP indirect DMA generates broken descriptors),
so the O(T*K*C) permutation+sum is applied to the device-computed
y/idx tensors host-side. All matmul FLOPs (distance matrix + conv) and
the top-k run on device.
"""

import sys
import numpy as np

if "/opt/trn_rl_repo" not in sys.path:
    sys.path.insert(0, "/opt/trn_rl_repo")

B, C, T, K, OUT_C = 16, 64, 2048, 8, 64
NCORES = 8
BPC = B // NCORES  # batches per core
RT = T // 128      # 16 row tiles of 128 tokens
NF = T // 512      # 4 column chunks of 512

_CACHE = {}


def build_nc():
    import concourse.bacc as bacc
    import concourse.tile as tile
    import concourse.mybir as mybir

    dt = mybir.dt
    f32 = dt.float32
    bf16 = dt.bfloat16

    nc = bacc.Bacc(
        "TRN2", target_bir_lowering=False, debug=False, num_devices=NCORES
    )
    xp_d = nc.dram_tensor("xpair", [BPC, 128, T], bf16, kind="ExternalInput").ap()
    ra_d = nc.dram_tensor("ra", [BPC, 128, T], bf16, kind="ExternalInput").ap()
    rb_d = nc.dram_tensor("rb", [BPC, 128, T], bf16, kind="ExternalInput").ap()
    nm_d = nc.dram_tensor("nrmf", [BPC, 1, T], f32, kind="ExternalInput").ap()
    nb_d = nc.dram_tensor("nrmb", [BPC, 3, T], bf16, kind="ExternalInput").ap()
    wall_d = nc.dram_tensor("wall", [128, K * OUT_C], bf16, kind="ExternalInput").ap()
    y_d = nc.dram_tensor("yout", [BPC, T, K * OUT_C], f32, kind="ExternalOutput").ap()
    gi_d = nc.dram_tensor("gidx", [BPC, T, K], dt.uint16, kind="ExternalOutput").ap()

    with tile.TileContext(nc) as tc:
        with (
            tc.tile_pool(name="const", bufs=1) as constp,
            tc.tile_pool(name="xio", bufs=2) as xio,
            tc.tile_pool(name="scoresp", bufs=4) as scp,
            tc.tile_pool(name="small", bufs=3) as smp,
            tc.tile_pool(name="yio", bufs=3) as yp,
            tc.tile_pool(name="pd", bufs=6, space="PSUM") as pdp,
            tc.tile_pool(name="py", bufs=2, space="PSUM") as pyp,
        ):
            wall_sb = constp.tile([128, K * OUT_C], bf16)
            nc.sync.dma_start(wall_sb[:], wall_d[:])
            ones3 = constp.tile([3, 128], bf16)
            nc.gpsimd.memset(ones3[:], 1.0)

            H = T // 2
            xps, ras, rbs, nms, nmbs = [], [], [], [], []
            for b in range(BPC):
                # split the big inputs into halves so the first tile's
                # matmuls (chunks 2,3 = second half) start after ~1/2 the
                # input DMA; prefetch BOTH batches before any compute.
                xpB = xio.tile([128, H], bf16, tag="xpB", name=f"xpB{b}")
                raB = xio.tile([128, H], bf16, tag="raB", name=f"raB{b}")
                xpA = xio.tile([128, H], bf16, tag="xpA", name=f"xpA{b}")
                raA = xio.tile([128, H], bf16, tag="raA", name=f"raA{b}")
                rbA = xio.tile([128, H], bf16, tag="rbA", name=f"rbA{b}")
                rbB = xio.tile([128, H], bf16, tag="rbB", name=f"rbB{b}")
                nm = xio.tile([3, T], bf16, tag="nm", name=f"nm{b}")
                nr = xio.tile([1, T], f32, tag="nr", name=f"nr{b}")
                nc.sync.dma_start(xpA[:], xp_d[b, :, 0:H])
                nc.sync.dma_start(rbA[:], rb_d[b, :, 0:H])
                nc.sync.dma_start(raB[:], ra_d[b, :, H:T])
                nc.sync.dma_start(xpB[:], xp_d[b, :, H:T])
                nc.sync.dma_start(nr[:], nm_d[b])
                nc.sync.dma_start(raA[:], ra_d[b, :, 0:H])
                nc.sync.dma_start(nm[:], nb_d[b])
                nc.sync.dma_start(rbB[:], rb_d[b, :, H:T])
                nmb = xio.tile([128, H], f32, tag="nmb", name=f"nmb{b}")
                nc.gpsimd.partition_broadcast(nmb[:], nr[:, H:])
                xps.append((xpA, xpB))
                ras.append((raA, raB))
                rbs.append((rbA, rbB))
                nms.append(nm)
                nmbs.append(nmb)

            for b in range(BPC):
                (xpA, xpB), (raA, raB), (rbA, rbB) = xps[b], ras[b], rbs[b]
                nm, nmb = nms[b], nmbs[b]
                for rt in range(RT):
                    ts = slice((rt * 128) % H, (rt * 128) % H + 128)
                    lhs = (xpA if rt < RT // 2 else xpB)[:, ts]
                    lhs2 = (rbA if rt < RT // 2 else rbB)[:, ts]
                    scores = scp.tile([128, T], f32, tag="scores", name=f"sc{b}_{rt}")
                    # chunks 2,3 first (no PE norm matmul; gpsimd adds -norm)
                    for nf in (2, 3, 0, 1):
                        cs = slice(nf * 512, (nf + 1) * 512)
                        hs = slice((nf * 512) % H, (nf * 512) % H + 512)
                        rav = (raA if nf < 2 else raB)[:, hs]
                        xpv = (xpA if nf < 2 else xpB)[:, hs]
                        pd = pdp.tile([128, 512], f32, tag="pd", name=f"pd{b}_{rt}_{nf}")
                        if nf >= 2:
                            nc.tensor.matmul(pd[:], lhs, rav, start=True, stop=False)
                            nc.tensor.matmul(pd[:], lhs2, xpv, start=False, stop=True)
                        else:
                            nc.tensor.matmul(pd[:], lhs, rav, start=True, stop=False)
                            nc.tensor.matmul(pd[:], lhs2, xpv, start=False, stop=False)
                            nc.tensor.matmul(
                                pd[:], ones3[:], nm[:, cs], start=False, stop=True
                            )
                        nc.scalar.copy(scores[:, cs], pd[:])
                        if nf == 3:
                            nc.gpsimd.tensor_add(
                                scores[:, H:], scores[:, H:], nmb[:]
                            )

                    vals = smp.tile([128, 8], f32, tag="vals", name=f"v{b}_{rt}")
                    nc.vector.max(vals[:], scores[:])
                    gall = smp.tile([128, 8], dt.uint16, tag="gall", name=f"g{b}_{rt}")
                    nc.vector.max_index(gall[:], vals[:], scores[:])
                    nc.sync.dma_start(gi_d[b, rt * 128 : (rt + 1) * 128, :], gall[:])

                    py = pyp.tile([128, 512], f32, tag="py", name=f"py{b}_{rt}")
                    nc.tensor.matmul(py[:], lhs, wall_sb[:])
                    ysb = yp.tile([128, 512], f32, tag="ysb", name=f"y{b}_{rt}")
                    nc.scalar.copy(ysb[:], py[:])
                    nc.sync.dma_start(y_d[b, rt * 128 : (rt + 1) * 128, :], ysb[:])

    nc.compile()
    return nc


def _get_nc():
    if "nc" not in _CACHE:
        _CACHE["nc"] = build_nc()
    return _CACHE["nc"]


def host_inputs(x, w, b):
    """Per-core input maps from full inputs."""
    import ml_dtypes

    bf = ml_dtypes.bfloat16
    x = np.asarray(x, dtype=np.float32)
    w = np.asarray(w, dtype=np.float32)
    b = np.asarray(b, dtype=np.float32)

    xh = x.astype(bf).astype(np.float32)
    xl = (x - xh).astype(bf).astype(np.float32)
    norm = (x.astype(np.float64) ** 2).sum(axis=1).astype(np.float32)  # [B, T]
    nhi = norm.astype(bf).astype(np.float32)
    nmid = (norm - nhi).astype(bf).astype(np.float32)
    nlo = (norm - nhi - nmid).astype(bf).astype(np.float32)

    xpair = np.concatenate([xh, xl], axis=1).astype(bf)           # [B, 128, T]
    ra = np.concatenate([2 * xh, 2 * xl], axis=1).astype(bf)      # [B, 128, T]
    rb = np.concatenate([2 * xl, 2 * xh], axis=1).astype(bf)      # [B, 128, T]
    nrmf = (-norm)[:, None, :].astype(np.float32)                 # [B, 1, T]
    nrmb = np.stack([-nhi, -nmid, -nlo], axis=1).astype(bf)       # [B, 3, T]

    wr = w.transpose(1, 2, 0).reshape(C, K * OUT_C)               # [c, j*64+o]
    wall = np.concatenate([wr, wr], axis=0).astype(bf)            # [128, 512]

    return [
        {
            "xpair": np.ascontiguousarray(xpair[i * BPC : (i + 1) * BPC]),
            "ra": np.ascontiguousarray(ra[i * BPC : (i + 1) * BPC]),
            "rb": np.ascontiguousarray(rb[i * BPC : (i + 1) * BPC]),
            "nrmf": np.ascontiguousarray(nrmf[i * BPC : (i + 1) * BPC]),
            "nrmb": np.ascontiguousarray(nrmb[i * BPC : (i + 1) * BPC]),
            "wall": wall,
        }
        for i in range(NCORES)
    ]


def kernel(x, w, b):
    from concourse.bass_utils import run_bass_kernel_spmd

    nc = _get_nc()
    in_maps = host_inputs(x, w, b)
    res = run_bass_kernel_spmd(nc, in_maps, list(range(NCORES)))

    b32 = np.asarray(b, dtype=np.float32)
    out = np.empty((B, OUT_C, T), np.float32)
    jj = np.arange(K, dtype=np.int64)[None, :]
    for i in range(NCORES):
        yv = res.results[i]["yout"]    # [BPC, T, K*OUT_C]
        gi = res.results[i]["gidx"]    # [BPC, T, K] u16
        for bb in range(BPC):
            idx = gi[bb].astype(np.int64)                 # [T, K]
            yr = yv[bb].reshape(T, K, OUT_C)              # [s, j, o]
            gathered = yr[idx, jj, :]                     # [T, K, OUT_C]
            out[i * BPC + bb] = gathered.sum(1).T + b32[:, None]
    return out.astype(np.float32)
